# revision 49
# baseline (speedup 1.0000x reference)
"""Trainium2 Bass kernel for nn_EmberBlock (dense transformer block with LIF-gated
attention). 8-core SPMD: head-parallel attention (4 heads/core, one batch per
4-core group) + sequence-parallel MLP after per-slab ReduceScatter chunks.

kernel(**inputs) takes FULL unsharded inputs (as in reference.setup_inputs())
and returns the FULL [B, T, C] output.

Key optimizations over the naive phased version (851us -> ~555us):
- h^T and V^T transposes on the DMA xbar (dma_start_transpose); m^T stays on
  the PE because Tile serializes DMA transposes with collectives.
- LIF gate fused into one scalar_tensor_tensor: m_un = (tanh + c) * e with
  free accumulation of the renorm sum. The per-head factor A=(1-lk)/2 cancels
  in m_un / sum(m_un).
- Causal masking via an additive -30 mask accumulated into the S PSUM group
  by the PE itself (identity.T @ mask), so exp runs in large chunks with no
  vector-engine op in the S->exp chain.
- ReduceScatter in 4 per-slab bf16 chunks on separate DRAM tensors; chunks
  0-2 complete under attention compute.
- MLP fc in fp8e4m3 DoubleRow (weights pre-scaled x64, unscaled in the gelu);
  proj stays bf16 to hold rel_err at 1.73e-2 under the 2e-2 gate.
- MLP split by token tiles: fc/proj for tiles 0-2 start at phase-E entry and
  fully hide the last RS chunk; tile 3 finishes afterwards.
- Engine placement tuned throughout: PSUM->SBUF copies on the DVE, o2 stores
  and transposes spread across the two hardware DGE rings, QKV's first n-slab
  split so the PE starts after two LayerNorm tiles.
"""
import numpy as np
import ml_dtypes

import concourse.bass as bass
import concourse.mybir as mybir
import concourse.tile as tile
from concourse import bacc
from concourse.bass_utils import run_bass_kernel_spmd

F32 = mybir.dt.float32
FP8 = mybir.dt.float8e4
BF16 = mybir.dt.bfloat16
AF = mybir.ActivationFunctionType
ALU = mybir.AluOpType

# model dims (hardcoded per spec)
B, T, C = 2, 2048, 1024
H, D = 16, 64
FF = 4 * C                    # 4096
N_CORES = 8
GROUP = 4                     # cores per batch
HL = H // GROUP               # 4 local heads
LC = HL * D                   # 256 local head feature cols
EPS_LN = 1e-5
P = 128                       # partitions
NT = T // P                   # 16 token tiles per batch
NQS = 4                       # q-slabs of 512 tokens
MLP_TOK = 512                 # tokens per core in MLP phase (4 tiles of 128)
W8SCALE = 64.0                # fp8 weight pre-scale for wfc/wmlp

_CACHED_NC = None


def _build():
    nc = bacc.Bacc(None, target_bir_lowering=False, debug=False, num_devices=N_CORES)

    # ---------------- I/O ----------------
    x_b = nc.dram_tensor("x_b", [T, C], BF16, kind="ExternalInput")
    x_res = nc.dram_tensor("x_res", [MLP_TOK, C], F32, kind="ExternalInput")
    wqkv = nc.dram_tensor("wqkv", [C, 3 * LC], BF16, kind="ExternalInput")
    bqkv = nc.dram_tensor("bqkv", [3 * LC], F32, kind="ExternalInput")
    wproj = nc.dram_tensor("wproj", [LC, C], BF16, kind="ExternalInput")
    bproj = nc.dram_tensor("bproj", [C], F32, kind="ExternalInput")
    wfc = nc.dram_tensor("wfc", [C, FF], FP8, kind="ExternalInput")
    bfc = nc.dram_tensor("bfc", [FF], F32, kind="ExternalInput")
    wmlp = nc.dram_tensor("wmlp", [FF, C], BF16, kind="ExternalInput")
    bmlp = nc.dram_tensor("bmlp", [C], F32, kind="ExternalInput")
    # lif rows: 0: st/(2) (tanh scale pre-div by se), 1: -st*th/2 (tanh bias),
    #           2: c = (1+lk)/(1-lk) (gate offset)
    lif = nc.dram_tensor("lif", [4, HL], F32, kind="ExternalInput")
    out = nc.dram_tensor("out", [MLP_TOK, C], F32, kind="ExternalOutput")

    # RS bounce buffers (internal DRAM), one pair per q-slab so the chunked
    # ReduceScatters carry no false whole-tensor WAR dependencies
    rs_in = [nc.dram_tensor(f"rs_in{i}", [512, C], BF16) for i in range(NQS)]
    rs_out = [nc.dram_tensor(f"rs_out{i}", [P, C], BF16) for i in range(NQS)]

    # constants embedded in the NEFF: additive causal mask (0 on/below diag,
    # -30 above) for the diagonal S tile, in f32 for PSUM add
    mask_np = np.where(np.tril(np.ones((P, P), np.float32)) > 0, 0.0,
                       -30.0).astype(np.float32)
    mask_c = nc.inline_tensor(mask_np.astype(ml_dtypes.bfloat16), name="mask_c")
    id_bf = nc.inline_tensor(np.eye(P, dtype=ml_dtypes.bfloat16), name="id_bf")

    replica_groups = [[0, 1, 2, 3], [4, 5, 6, 7]]

    from contextlib import ExitStack
    with tile.TileContext(nc) as tc, ExitStack() as root_ctx:
        attn_ctx = ExitStack()  # pools freed after attention

        consts = root_ctx.enter_context(tc.tile_pool(name="consts", bufs=1))
        zero_c = consts.tile([P, 1], F32)
        nc.vector.memset(zero_c[:], 0.0)
        nc.const_aps.aps[(F32, 0.0)] = zero_c[:]
        eps_c = consts.tile([P, 1], F32)
        nc.vector.memset(eps_c[:], EPS_LN)
        nc.const_aps.aps[(F32, EPS_LN)] = eps_c[:]
        maskt = consts.tile([P, P], BF16)
        nc.sync.dma_start(out=maskt[:], in_=mask_c[:, :])
        ident = consts.tile([P, P], BF16)
        nc.sync.dma_start(out=ident[:], in_=id_bf[:, :])
        # per-head LIF constants broadcast to all partitions: [128, 4, HL]
        lif_sb = consts.tile([P, 4, HL], F32)
        nc.sync.dma_start(out=lif_sb[:], in_=lif[None, :, :].to_broadcast((P, 4, HL)))
        # biases in per-partition layout
        bqkv_sb = consts.tile([P, 6], F32)
        nc.sync.dma_start(out=bqkv_sb[:], in_=bqkv.rearrange("(t p) -> p t", p=P))
        bfc_sb = consts.tile([P, FF // P], F32)
        nc.sync.dma_start(out=bfc_sb[:], in_=bfc.rearrange("(t p) -> p t", p=P))
        # free-dim biases broadcast across partitions
        bproj_sb = consts.tile([P, C], BF16)
        nc.gpsimd.dma_start(out=bproj_sb[:], in_=bproj[None, :].to_broadcast((P, C)))
        bmlp_sb = consts.tile([P, C], BF16)
        nc.gpsimd.dma_start(out=bmlp_sb[:], in_=bmlp[None, :].to_broadcast((P, C)))
        # attention-projection weights (2 k-tiles), small -> consts
        wproj_sb = consts.tile([P, 2, C], BF16)
        for kt in range(2):
            nc.sync.dma_start(out=wproj_sb[:, kt, :], in_=wproj[kt * P:(kt + 1) * P, :])

        # =========== Phase A: LN1 over all T tokens + h^T (DMA transpose) ====
        xpool = root_ctx.enter_context(tc.tile_pool(name="xpool", bufs=2))
        stat_pool = root_ctx.enter_context(tc.tile_pool(name="stats", bufs=4))
        mm_psum = root_ctx.enter_context(tc.tile_pool(name="mm_psum", bufs=2, space="PSUM"))
        tp_psum = root_ctx.enter_context(tc.tile_pool(name="tp_psum", bufs=2, space="PSUM"))
        smallp = root_ctx.enter_context(tc.tile_pool(name="smallp", bufs=16))

        attn = attn_ctx.enter_context(tc.tile_pool(name="attn", bufs=1))
        epool = attn_ctx.enter_context(tc.tile_pool(name="epool", bufs=2))
        e4pool = attn_ctx.enter_context(tc.tile_pool(name="e4pool", bufs=7))
        spool = attn_ctx.enter_context(tc.tile_pool(name="spool", bufs=3, space="PSUM"))
        mtpool = attn_ctx.enter_context(tc.tile_pool(name="mtpool", bufs=2))
        ypool = attn_ctx.enter_context(tc.tile_pool(name="ypool", bufs=2))
        yps_pool = attn_ctx.enter_context(tc.tile_pool(name="yps", bufs=1, space="PSUM"))
        o2pool = attn_ctx.enter_context(tc.tile_pool(name="o2pool", bufs=2))
        p12 = attn_ctx.enter_context(tc.tile_pool(name="p12", bufs=1))

        # hT layout: [feat_p, tok_tile, feat_tile, tok] so each per-token-tile
        # DMA transpose writes a contiguous [128, 8, 128] block.
        hT = p12.tile([P, NT, C // P, P], BF16)
        wqkv_sb = p12.tile([P, 8, 3 * LC], BF16)  # 8 k-tiles of wqkv
        for kt in range(8):
            nc.sync.dma_start(out=wqkv_sb[:, kt, :], in_=wqkv[kt * P:(kt + 1) * P, :])

        def layernorm_tile(x_tile, h_out, tag):
            """x_tile [128, C] f32 -> h_out [128, C] bf16 (normalized, no affine)."""
            stats = stat_pool.tile([P, 2, 6], F32, name=f"st_{tag}")
            nc.vector.bn_stats(out=stats[:, 0, :], in_=x_tile[:, 0:512])
            nc.vector.bn_stats(out=stats[:, 1, :], in_=x_tile[:, 512:1024])
            mv = stat_pool.tile([P, 2], F32, name=f"mv_{tag}")
            nc.vector.bn_aggr(out=mv[:], in_=stats[:])
            std = stat_pool.tile([P, 1], F32, name=f"sd_{tag}")
            nc.scalar.activation(std[:], mv[:, 1:2], AF.Sqrt, bias=EPS_LN)
            rstd = stat_pool.tile([P, 1], F32, name=f"rs_{tag}")
            nc.vector.reciprocal(rstd[:], std[:])
            nmr = stat_pool.tile([P, 1], F32, name=f"nm_{tag}")
            nc.vector.tensor_scalar(out=nmr[:], in0=mv[:, 0:1], scalar1=rstd[:],
                                    scalar2=-1.0, op0=ALU.mult, op1=ALU.mult)
            nc.scalar.activation(h_out, x_tile, AF.Identity, bias=nmr[:], scale=rstd[:])

        for tt in range(NT):
            x_tile = xpool.tile([P, C], BF16, name="x_t", tag="xb_t")
            nc.sync.dma_start(out=x_tile[:], in_=x_b[tt * P:(tt + 1) * P, :])
            h_tile = xpool.tile([P, C], BF16, name="h_t", tag="h_t")
            layernorm_tile(x_tile[:], h_tile[:], f"ln1_{tt}")
            # hT[p, tt, ft, t] = h[t, ft*128+p]; alternate DGE rings so
            # consecutive tiles' transposes run on two DMA engines
            eng = nc.scalar if tt % 2 == 0 else nc.sync
            eng.dma_start_transpose(out=hT[:, tt, :, :], in_=h_tile[:])

        # =========== Phase B: QKV^T = wqkv^T @ h (feature-major) ===========
        qkT = attn.tile([P, 4, T], BF16)    # q (2 feat tiles) + k (2 feat tiles)
        vT = p12.tile([P, 2, T], BF16)      # v rows, freed after v_tok built
        for ns in range(4):
            for mt in range(6):
                dst = qkT if mt < 4 else vT
                dcol = mt if mt < 4 else mt - 4
                ps = mm_psum.tile([P, 512], F32, name="qkv_ps", tag="qkv_ps")
                nhalf = 2 if ns == 0 else 1
                for hf in range(nhalf):
                    t0 = 4 * ns + hf * (4 // nhalf)
                    t1 = t0 + 4 // nhalf
                    for kt in range(8):
                        nc.tensor.matmul(ps[:, hf * 256:hf * 256 + (t1 - t0) * P],
                                         wqkv_sb[:, kt, mt * P:(mt + 1) * P],
                                         hT[:, t0:t1, kt, :],
                                         start=(kt == 0), stop=(kt == 7))
                nc.scalar.activation(dst[:, dcol, ns * 512:(ns + 1) * 512], ps[:],
                                     AF.Identity, bias=bqkv_sb[:, mt:mt + 1])

        # =========== Phase C: V^T -> V (token-major) via DMA transpose =======
        # v_tok[p, vt, kb, c] = vT[c, vt, kb*128+p] = v[tok=kb*128+p, feat=vt*128+c]
        v_tok = attn.tile([P, 2, NT, P], BF16)
        for vt in range(2):
            eng = nc.sync if vt == 0 else nc.scalar
            eng.dma_start_transpose(
                out=v_tok[:, vt, :, :], in_=vT[:, vt, :])

        # =========== Phase D: attention per q-slab, per head ===========
        for qs in range(NQS):
            yT = [ypool.tile([P, 512], BF16, name=f"yT{i}_{qs}", tag=f"yT{i}")
                  for i in range(2)]  # [2 tiles of 128 feats][512 q] local Y^T
            state = {}

            def pass1(h):
                """S matmuls + masked exp with accumulated row-sums -> e_t, sprh."""
                qrow = (h % 2) * D
                qtile_idx = h // 2
                ktile_idx = 2 + h // 2
                e_ts, sprhs = {}, {}
                for qt in range(qs * 4, qs * 4 + 4):
                    j = qt - qs * 4
                    W = (qt + 1) * P
                    e_t = e4pool.tile([P, T], BF16, name="e_t", tag="e_t")
                    e_ts[j] = e_t
                    se_parts = smallp.tile([P, 4], F32, name="sep", tag="sep")
                    nch = (W + 511) // 512
                    for cs in range(nch):
                        w0 = cs * 512
                        w1 = min(w0 + 512, W)
                        ps = spool.tile([P, 512], F32, name="s_ps", tag="s_ps")
                        diag = w1 == W
                        nc.tensor.matmul(
                            ps[:, 0:w1 - w0],
                            qkT[qrow:qrow + D, qtile_idx, qt * P:(qt + 1) * P],
                            qkT[qrow:qrow + D, ktile_idx, w0:w1],
                            start=True, stop=not diag)
                        if diag:
                            # additive causal mask on the diagonal 128 cols,
                            # accumulated by the PE (identity.T @ mask = mask)
                            nc.tensor.matmul(
                                ps[:, W - 128 - w0:W - w0],
                                ident[:], maskt[:],
                                start=False, stop=True)
                        nc.scalar.activation(e_t[:, w0:w1], ps[:, 0:w1 - w0],
                                             AF.Exp,
                                             accum_out=se_parts[:, cs:cs + 1])
                    rp = smallp.tile([P, 1], F32, name="rp", tag="rp")
                    if nch > 1:
                        se = smallp.tile([P, 1], F32, name="se", tag="se")
                        nc.vector.reduce_sum(se[:], se_parts[:, 0:nch],
                                             axis=mybir.AxisListType.X)
                        nc.vector.reciprocal(rp[:], se[:])
                    else:
                        nc.vector.reciprocal(rp[:], se_parts[:, 0:1])
                    sprh = smallp.tile([P, 1], F32, name="sprh", tag="sprh")
                    nc.vector.tensor_scalar(out=sprh[:], in0=rp[:],
                                            scalar1=lif_sb[:, 0, h:h + 1],
                                            scalar2=None, op0=ALU.mult)
                    sprhs[j] = sprh
                state[h] = (e_ts, sprhs)

            def pass2(h):
                """tanh gate + fused m_un & row-sum + renorm + DMA-transpose + PV."""
                qrow = (h % 2) * D
                e_ts, sprhs = state.pop(h)
                # mT layout: [k_p, q_tile, k_tile, q] -> per-q-tile contiguous
                mT = mtpool.tile([P, 4, 16, P], BF16, name="mT", tag="mT")
                for qt in range(qs * 4, qs * 4 + 4):
                    j = qt - qs * 4
                    W = (qt + 1) * P
                    e_t = e_ts[j]
                    f_t = epool.tile([P, T], BF16, name="f_t", tag="f_t")
                    nc.scalar.activation(f_t[:, :W], e_t[:, :W], AF.Tanh,
                                         bias=lif_sb[:, 1, h:h + 1],
                                         scale=sprhs[j][:])
                    m_t = epool.tile([P, T], BF16, name="m_t", tag="m_t")
                    sm = smallp.tile([P, 1], F32, name="smc", tag="smc")
                    nc.vector.scalar_tensor_tensor(
                        out=m_t[:, :W], in0=f_t[:, :W],
                        scalar=lif_sb[:, 2, h:h + 1], in1=e_t[:, :W],
                        op0=ALU.add, op1=ALU.mult,
                        accum_out=sm[:])
                    rsm = smallp.tile([P, 1], F32, name="rsmc", tag="rsmc")
                    nc.vector.reciprocal(rsm[:], sm[:])
                    nc.vector.tensor_scalar(out=m_t[:, :W], in0=m_t[:, :W],
                                            scalar1=rsm[:],
                                            scalar2=None, op0=ALU.mult)
                    # transpose m [q, k] -> mT[k_p, j, kb, q] on the PE,
                    # PSUM->SBUF copies on the vector engine
                    for kg in range((qt + 1 + 3) // 4):
                        k0 = kg * 4
                        kn = min(4, qt + 1 - k0)
                        tp = tp_psum.tile([P, 4, P], BF16, name="mtp", tag="mtp")
                        for kk in range(kn):
                            nc.tensor.transpose(tp[:, kk, :],
                                                m_t[:, (k0 + kk) * P:(k0 + kk + 1) * P],
                                                ident[:])
                        nc.vector.tensor_copy(out=mT[:, j, k0:k0 + kn, :],
                                              in_=tp[:, 0:kn, :])
                # PV: yps[d, q] accumulated over kb; rhs strided over q-tiles
                yps = yps_pool.tile([D, 512], F32, name="yps", tag="yps")
                nkb = qs * 4 + 4
                for kb in range(nkb):
                    j0 = max(0, kb - qs * 4)
                    nc.tensor.matmul(yps[:, j0 * P:512],
                                     v_tok[:, h // 2, kb,
                                           (h % 2) * D:(h % 2) * D + D],
                                     mT[:, j0:4, kb, :],
                                     start=(kb == 0), stop=(kb == nkb - 1))
                nc.vector.tensor_copy(out=yT[h // 2][qrow:qrow + D, :], in_=yps[:])

            # two-deep cross-head pipeline
            pass1(0)
            for h in range(1, HL):
                pass1(h)
                pass2(h - 1)
            pass2(HL - 1)

            # attn-proj for this q-slab: out2 = Y @ wproj  (token-major)
            for mt in range(4):
                o2 = o2pool.tile([P, C], BF16, name="o2", tag="o2")
                for ns in range(2):
                    ps = mm_psum.tile([P, 512], F32, name="o2_ps", tag="qkv_ps")
                    for kt in range(2):
                        nc.tensor.matmul(ps[:],
                                         yT[kt][:, mt * P:(mt + 1) * P],
                                         wproj_sb[:, kt, ns * 512:(ns + 1) * 512],
                                         start=(kt == 0), stop=(kt == 1))
                    nc.vector.tensor_copy(out=o2[:, ns * 512:(ns + 1) * 512],
                                          in_=ps[:])
                nc.sync.dma_start(
                    out=rs_in[qs][mt * P:(mt + 1) * P, :], in_=o2[:])

            # per-slab ReduceScatter chunk (overlaps later slabs' compute)
            nc.gpsimd.collective_compute(
                "ReduceScatter", ALU.add, replica_groups=replica_groups,
                ins=[rs_in[qs][:, :]],
                outs=[rs_out[qs][:, :]])

        # release attention pools
        attn_ctx.close()

        # =========== Phase E: MLP on 512 local tokens (feature-major) ========
        mlp = root_ctx.enter_context(tc.tile_pool(name="mlp", bufs=1))
        wstream = root_ctx.enter_context(tc.tile_pool(name="wstream", bufs=2))
        wfc_sb = mlp.tile([P, 8, FF], FP8)
        h2T = mlp.tile([P, 8, 4, P], FP8)   # [feat_p, feat_tile, tok_tile, tok]
        x1_t = {}

        def ln2_chain(j):
            rs_sb = xpool.tile([P, C], BF16, name="rs_sb", tag="h_t")
            nc.sync.dma_start(out=rs_sb[:], in_=rs_out[j][:, :])
            xr = xpool.tile([P, C], F32, name="xr", tag="x_t")
            nc.sync.dma_start(out=xr[:], in_=x_res[j * P:(j + 1) * P, :])
            x1 = mlp.tile([P, C], F32, name=f"x1_{j}")
            nc.vector.tensor_tensor(out=x1[:], in0=xr[:], in1=rs_sb[:], op=ALU.add)
            nc.vector.tensor_tensor(out=x1[:], in0=x1[:], in1=bproj_sb[:], op=ALU.add)
            x1_t[j] = x1
            h2 = xpool.tile([P, C], BF16, name="h2", tag="h_t")
            layernorm_tile(x1[:], h2[:], f"ln2_{j}")
            for fg in range(2):
                tp = tp_psum.tile([P, 4, P], BF16, name="h2tp", tag="mtp")
                for k in range(4):
                    ft = fg * 4 + k
                    nc.tensor.transpose(tp[:, k, :], h2[:, ft * P:(ft + 1) * P],
                                        ident[:])
                nc.vector.tensor_copy(out=h2T[:, fg * 4:(fg + 1) * 4, j, :],
                                      in_=tp[:])

        # token tiles 0-2 first: their RS chunks completed during attention
        for j in range(3):
            ln2_chain(j)
        for kt in range(8):
            nc.sync.dma_start(out=wfc_sb[:, kt, :], in_=wfc[kt * P:(kt + 1) * P, :])

        aT = mlp.tile([P, FF // P, MLP_TOK], BF16)
        o_sb_t = [mlp.tile([P, C], F32, name=f"o_sb{j}") for j in range(4)]

        def fc(c0, c1, jlo, jhi):
            """fc+gelu (DoubleRow fp8) over token cols [c0:c1] = tiles jlo:jhi."""
            for mt in range(FF // P):
                ps = mm_psum.tile([P, 512], F32, name="fc_ps", tag="qkv_ps")
                for ks in range(4):
                    nc.tensor.matmul(ps[:, 0:c1 - c0],
                                     wfc_sb[:, 2 * ks:2 * ks + 2, mt * P:(mt + 1) * P],
                                     h2T[:, 2 * ks:2 * ks + 2, jlo:jhi, :],
                                     start=(ks == 0), stop=(ks == 3),
                                     perf_mode=mybir.MatmulPerfMode.DoubleRow)
                nc.scalar.activation(aT[:, mt, c0:c1], ps[:, 0:c1 - c0], AF.Gelu,
                                     bias=bfc_sb[:, mt:mt + 1], scale=1.0 / W8SCALE)

        def proj(wm_half, ns, js):
            for j in js:
                ps = mm_psum.tile([P, 512], F32, name="o3_ps", tag="qkv_ps")
                for kt in range(FF // P):
                    nc.tensor.matmul(ps[:], aT[:, kt, j * P:(j + 1) * P],
                                     wm_half[:, kt, :],
                                     start=(kt == 0), stop=(kt == FF // P - 1))
                nc.vector.tensor_tensor(out=o_sb_t[j][:, ns * 512:(ns + 1) * 512],
                                        in0=x1_t[j][:, ns * 512:(ns + 1) * 512],
                                        in1=ps[:], op=ALU.add)

        def finish(j):
            nc.vector.tensor_tensor(out=o_sb_t[j][:], in0=o_sb_t[j][:],
                                    in1=bmlp_sb[:], op=ALU.add)
            nc.sync.dma_start(out=out[j * P:(j + 1) * P, :], in_=o_sb_t[j][:])

        # fc+proj for tiles 0-2 while RS3 is still in flight
        fc(0, 384, 0, 3)
        wm_halves = []
        for ns in range(2):
            wm_half = wstream.tile([P, FF // P, 512], BF16, name="wm_h", tag="wm_h")
            for kt in range(FF // P):
                nc.sync.dma_start(out=wm_half[:, kt, :],
                                  in_=wmlp[kt * P:(kt + 1) * P,
                                           ns * 512:(ns + 1) * 512])
            wm_halves.append(wm_half)
            proj(wm_half, ns, range(3))
        for j in range(3):
            finish(j)
        # tile 3: RS3 has long completed -> queues never block on it
        ln2_chain(3)
        fc(384, 512, 3, 4)
        for ns in range(2):
            proj(wm_halves[ns], ns, [3])
        finish(3)

    nc.compile()
    return nc


def _get_nc():
    global _CACHED_NC
    if _CACHED_NC is None:
        _CACHED_NC = _build()
    return _CACHED_NC


def _softplus(x):
    return np.log1p(np.exp(-np.abs(x))) + np.maximum(x, 0.0)


def _bf16(x):
    return np.ascontiguousarray(x.astype(ml_dtypes.bfloat16))


def kernel(x, ln1_w, ln1_b, w_attn, b_attn, w_attn_proj, b_attn_proj,
           threshold, leak, steepness, ln2_w, ln2_b,
           w_fc, b_fc, w_mlp_proj, b_mlp_proj):
    x = np.asarray(x, np.float32)
    f32 = lambda a: np.asarray(a, np.float32)
    ln1_w, ln1_b, w_attn, b_attn = map(f32, (ln1_w, ln1_b, w_attn, b_attn))
    w_attn_proj, b_attn_proj = f32(w_attn_proj), f32(b_attn_proj)
    threshold, leak, steepness = map(f32, (threshold, leak, steepness))
    ln2_w, ln2_b, w_fc, b_fc = map(f32, (ln2_w, ln2_b, w_fc, b_fc))
    w_mlp_proj, b_mlp_proj = f32(w_mlp_proj), f32(b_mlp_proj)

    # fold LN affine into the following matmuls (exact in fp32 algebra)
    wa = w_attn * ln1_w[:, None]
    ba = b_attn + ln1_b @ w_attn
    # fold 1/sqrt(D) into the q columns
    wa = wa.copy()
    wa[:, :C] *= 1.0 / np.sqrt(D)
    ba = ba.copy()
    ba[:C] *= 1.0 / np.sqrt(D)
    wf = w_fc * ln2_w[:, None]
    bf = b_fc + ln2_b @ w_fc

    # per-head LIF constants.  With A = (1-lk)/2, c = (1+lk)/(1-lk):
    #   m_un = (tanh(st/(2 se) e - st th/2) + c) * e = (w_gate * p) * se / A
    # and the renormalization cancels se / A exactly.
    st = _softplus(steepness)
    lk = 1.0 / (1.0 + np.exp(-leak))
    th = np.abs(threshold) * 0.1

    wf_b = np.ascontiguousarray((wf * W8SCALE).astype(ml_dtypes.float8_e4m3fn))
    wm_b = _bf16(w_mlp_proj)

    in_maps = []
    for cix in range(N_CORES):
        b = cix // GROUP
        r = cix % GROUP
        h0 = r * HL * D  # first local head feature col
        cols = (list(range(h0, h0 + LC))
                + list(range(C + h0, C + h0 + LC))
                + list(range(2 * C + h0, 2 * C + h0 + LC)))
        wqkv_local = _bf16(wa[:, cols])
        bqkv_local = np.ascontiguousarray(ba[cols], dtype=np.float32)
        wproj_local = _bf16(w_attn_proj[h0:h0 + LC, :])
        hsl = slice(r * HL, (r + 1) * HL)
        lif_local = np.stack([
            st[hsl] / 2.0,
            -(st[hsl] * th[hsl]) / 2.0,
            (1.0 + lk[hsl]) / (1.0 - lk[hsl]),
            np.zeros(HL, np.float32),
        ]).astype(np.float32)
        x_b_core = np.ascontiguousarray(x[b].astype(ml_dtypes.bfloat16))
        # MLP-phase tokens: RS chunk qs hands rank r rows qs*512+r*128..+128
        x_res_core = np.ascontiguousarray(np.concatenate(
            [x[b][qs * 512 + r * P: qs * 512 + (r + 1) * P] for qs in range(4)],
            axis=0))
        in_maps.append({
            "x_b": x_b_core,
            "x_res": x_res_core,
            "wqkv": wqkv_local,
            "bqkv": bqkv_local,
            "wproj": wproj_local,
            "bproj": b_attn_proj,
            "wfc": wf_b,
            "bfc": bf.astype(np.float32),
            "wmlp": wm_b,
            "bmlp": b_mlp_proj,
            "lif": lif_local,
        })

    global _last_in_maps
    _last_in_maps = in_maps
    nc = _get_nc()
    res = run_bass_kernel_spmd(nc, in_maps, list(range(N_CORES)))

    out = np.empty((B, T, C), np.float32)
    for cix in range(N_CORES):
        b = cix // GROUP
        r = cix % GROUP
        for qs in range(4):
            out[b, qs * 512 + r * P: qs * 512 + (r + 1) * P, :] = \
                res.results[cix]["out"][qs * P:(qs + 1) * P]
    return out
